# revision 1
# baseline (speedup 1.0000x reference)
"""ConvAttention Trainium2 kernel (self-contained).

Math: scores[b,h,w,t,s] = convQ(Q)[...,s] + convK(K)[...,t] + b2, softmax over t.
The s-dependent and constant terms cancel in the softmax, so
  attn[b,h,w,t] = softmax_t(A_k[b,h,w,t]),  A_k = conv5x5(K) (K = w1k x + b1k)
  out[b,c,h,w,s] = sum_c' wv[c,c'] * (sum_t attn[b,h,w,t] x[b,c',h,w,t]) + bv[c]
(independent of s, Q entirely). Sharding: data-parallel over batch, 1 batch/core.

Device layout per core: partitions = (hb in 2, c in 64), free = (h' in 32, w 64, t 12).
Stages: tap-map matmul (fused u = w2k.w1k) -> DRAM scatter roundtrip to
[h partitions, taps free] -> 25 shift-matrix conv matmuls -> softmax ->
DRAM replicate roundtrip -> p*x (DVE) -> 12 t-accumulated wv matmuls ->
bias drain -> s-broadcast store.
"""
import numpy as np
from contextlib import ExitStack

B, C, H, W, S = 8, 64, 64, 64, 12
K5 = 5
HB, HH = 2, 32          # h split into two halves across partition blocks
FREE = HH * W * S       # 24576 per partition
NT = 25                 # taps
WT = W * S              # 768
NCH = 4                 # stage-B chunks (h' rows per chunk = 8)
CHF = FREE // NCH       # 6144
CHW = HH * W // NCH     # 512 output cols per chunk

_cache = {}


def _build_program():
    import concourse.bass as bass
    import concourse.tile as tile
    from concourse import bacc, mybir
    f32 = mybir.dt.float32

    nc = bacc.Bacc("TRN2", target_bir_lowering=False, debug=False, num_devices=8)
    x_in = nc.dram_tensor("x", [128, FREE], f32, kind="ExternalInput")
    u2_in = nc.dram_tensor("u2", [128, 2 * NT], f32, kind="ExternalInput")
    s5_in = nc.dram_tensor("s5", [64, 5 * 64], f32, kind="ExternalInput")
    wv2_in = nc.dram_tensor("wv2", [128, 128], f32, kind="ExternalInput")
    b12_in = nc.dram_tensor("b12", [64, WT], f32, kind="ExternalInput")
    bv2_in = nc.dram_tensor("bv2", [128, 1], f32, kind="ExternalInput")
    y_out = nc.dram_tensor("y", [128, FREE], f32, kind="ExternalOutput")

    with tile.TileContext(nc) as tc:
        with ExitStack() as ctx:
            cpool = ctx.enter_context(tc.tile_pool(name="consts", bufs=1))
            u2 = cpool.tile([128, 2 * NT], f32)
            nc.sync.dma_start(u2[:], u2_in[:])
            s5 = cpool.tile([64, 5 * 64], f32)
            nc.sync.dma_start(s5[:], s5_in[:])
            wv2 = cpool.tile([128, 128], f32)
            nc.sync.dma_start(wv2[:], wv2_in[:])
            b12 = cpool.tile([64, WT], f32)
            nc.sync.dma_start(b12[:], b12_in[:])
            bv2 = cpool.tile([128, 1], f32)
            nc.sync.dma_start(bv2[:], bv2_in[:])

            xpool = ctx.enter_context(tc.tile_pool(name="x", bufs=1))
            x_sb = xpool.tile([128, FREE], f32)
            for j in range(8):
                sl = bass.ts(j, FREE // 8)
                nc.sync.dma_start(x_sb[:, sl], x_in[:, sl])

            dpool = ctx.enter_context(tc.tile_pool(name="dram", bufs=1, space="DRAM"))
            dram_m = dpool.tile([50, FREE], f32)
            dram_p = dpool.tile([64, WT], f32)

            ppool = ctx.enter_context(tc.tile_pool(name="p", bufs=1))
            p_sb = ppool.tile([64, WT], f32)

            # ---- stage A: tap maps + conv + softmax ----
            with ExitStack() as actx:
                ps1 = actx.enter_context(tc.tile_pool(name="ps1", bufs=4, space="PSUM"))
                mpp = actx.enter_context(tc.tile_pool(name="mp", bufs=4))
                NJ = 64
                for j in range(NJ):
                    ps = ps1.tile([50, 384], f32)
                    nc.tensor.matmul(ps[:], u2[:], x_sb[:, bass.ts(j, 384)],
                                     start=True, stop=True)
                    mp_t = mpp.tile([50, 384], f32)
                    nc.scalar.copy(mp_t[:], ps[:])
                    nc.sync.dma_start(dram_m[:, bass.ts(j, 384)], mp_t[:])

                dpoolD = actx.enter_context(tc.tile_pool(name="D", bufs=1))
                d_sb = dpoolD.tile([64, NT * WT], f32)
                # dram_m rows (hb,r), cols (h', f) -> D partitions (hb,h'), free (r,f)
                for hb in range(2):
                    src = dram_m[hb * NT:(hb + 1) * NT, :].rearrange(
                        "r (h f) -> r h f", h=HH).transpose([1, 0, 2])
                    nc.sync.dma_start(d_sb[hb * HH:(hb + 1) * HH, :], src)

                psA = actx.enter_context(tc.tile_pool(name="psA", bufs=2, space="PSUM"))
                apool = actx.enter_context(tc.tile_pool(name="asb", bufs=1))
                a_sb = apool.tile([64, WT], f32)
                smp = actx.enter_context(tc.tile_pool(name="smax", bufs=1))
                for wh in range(2):
                    a_ps = psA.tile([64, 384], f32)
                    order = [(dh, dw) for dh in range(5) for dw in [2, 0, 1, 3, 4]]
                    for i, (dh, dw) in enumerate(order):
                        r = dh * 5 + dw
                        lo = max(wh * 32, 2 - dw)
                        hi = min(wh * 32 + 32, 66 - dw)
                        rhs = d_sb[:, r * WT + (lo + dw - 2) * S: r * WT + (hi + dw - 2) * S]
                        out = a_ps[:, (lo - wh * 32) * S: (hi - wh * 32) * S]
                        nc.tensor.matmul(out, s5[:, bass.ts(dh, 64)], rhs,
                                         start=(i == 0), stop=(i == 24))
                    nc.vector.tensor_add(a_sb[:, bass.ts(wh, 384)], a_ps[:],
                                         b12[:, bass.ts(wh, 384)])
                a3 = a_sb[:].rearrange("p (w t) -> p w t", t=S)
                mx = smp.tile([64, W], f32)
                nc.vector.tensor_reduce(mx[:], a3, axis=mybir.AxisListType.X,
                                        op=mybir.AluOpType.max)
                nc.vector.tensor_sub(a3, a3, mx[:].broadcast_to([64, W, S]))
                nc.scalar.activation(a_sb[:], a_sb[:], mybir.ActivationFunctionType.Exp)
                sm = smp.tile([64, W], f32)
                nc.vector.tensor_reduce(sm[:], a3, axis=mybir.AxisListType.X,
                                        op=mybir.AluOpType.add)
                rcp = smp.tile([64, W], f32)
                nc.vector.reciprocal(rcp[:], sm[:])
                nc.vector.tensor_mul(p_sb[:].rearrange("p (w t) -> p w t", t=S),
                                     a3, rcp[:].broadcast_to([64, W, S]))
                nc.sync.dma_start(dram_p[:], p_sb[:])

            # ---- stage B: weight, project, store ----
            with ExitStack() as bctx:
                pwp = bctx.enter_context(tc.tile_pool(name="pw", bufs=2))
                psO = bctx.enter_context(tc.tile_pool(name="psO", bufs=2, space="PSUM"))
                op = bctx.enter_context(tc.tile_pool(name="osb", bufs=2))
                pr = dram_p[:].rearrange("(hb q) f -> hb (q f)", hb=2)
                for ch in range(NCH):
                    pw = pwp.tile([128, CHF], f32)
                    for hb in range(2):
                        nc.sync.dma_start(pw[hb * 64: hb * 64 + 1, :],
                                          pr[hb: hb + 1, bass.ts(ch, CHF)])
                        for k in range(6):
                            lo, n = hb * 64, 1 << k
                            nc.sync.dma_start(pw[lo + n: lo + 2 * n, :],
                                              pw[lo: lo + n, :])
                    nc.vector.tensor_mul(pw[:], x_sb[:, bass.ts(ch, CHF)], pw[:])
                    o_ps = psO.tile([128, CHW], f32)
                    pw3 = pw[:].rearrange("p (hw t) -> p hw t", t=S)
                    for t in range(S):
                        nc.tensor.matmul(o_ps[:], wv2[:], pw3[:, :, t: t + 1],
                                         start=(t == 0), stop=(t == S - 1))
                    o12 = op.tile([128, CHF], f32)
                    nc.scalar.activation(
                        o12[:].rearrange("p (hw s) -> p hw s", s=S),
                        o_ps[:].broadcast_to([128, CHW, S]),
                        mybir.ActivationFunctionType.Identity, bias=bv2[:, 0:1])
                    nc.sync.dma_start(y_out[:, bass.ts(ch, CHF)], o12[:])

    nc.compile()
    return nc


def _prep_weights(w1, b1, w2, b2):
    w1f = w1[:, :, 0, 0].astype(np.float32)
    wk, wv = w1f[64:128], w1f[128:192]
    b1k, bv = b1[64:128].astype(np.float32), b1[128:192].astype(np.float32)
    w2k = w2[0, 64:128].astype(np.float32)          # [c,5,5]
    u = np.tensordot(w2k, wk, axes=([0], [0])).reshape(NT, C)   # [25, 64]
    u2 = np.zeros((128, 2 * NT), np.float32)
    u2[0:64, 0:NT] = u.T
    u2[64:128, NT:2 * NT] = u.T
    beta = np.tensordot(w2k, b1k, axes=([0], [0]))  # [5,5]
    Bm = np.zeros((H, W), np.float32)
    for dh in range(5):
        hm = ((np.arange(H) + dh - 2 >= 0) & (np.arange(H) + dh - 2 < H))
        for dw in range(5):
            wm = ((np.arange(W) + dw - 2 >= 0) & (np.arange(W) + dw - 2 < W))
            Bm += beta[dh, dw] * np.outer(hm, wm).astype(np.float32)
    b12 = np.repeat(Bm, S, axis=1).astype(np.float32)           # [64, 768]
    s5 = np.zeros((64, 5, 64), np.float32)
    for dh in range(5):
        for h in range(64):
            hp = h + dh - 2
            if 0 <= hp < 64:
                s5[hp, dh, h] = 1.0
    s5 = s5.reshape(64, 5 * 64)
    wv2 = np.zeros((128, 128), np.float32)
    wv2[0:64, 0:64] = wv.T
    wv2[64:128, 64:128] = wv.T
    bv2 = np.concatenate([bv, bv]).reshape(128, 1).astype(np.float32)
    return u2, s5, wv2, b12, bv2


def _run(x, w1, b1, w2, b2, trace=False):
    from concourse.bass_utils import run_bass_kernel_spmd
    if "nc" not in _cache:
        _cache["nc"] = _build_program()
    nc = _cache["nc"]
    u2, s5, wv2, b12, bv2 = _prep_weights(w1, b1, w2, b2)
    in_maps = []
    for b in range(B):
        xb = np.ascontiguousarray(
            x[b].reshape(C, HB, HH, W * S).transpose(1, 0, 2, 3).reshape(128, FREE))
        in_maps.append({"x": xb, "u2": u2, "s5": s5, "wv2": wv2,
                        "b12": b12, "bv2": bv2})
    res = run_bass_kernel_spmd(nc, in_maps, core_ids=list(range(8)), trace=trace)
    out = np.empty((B, C, H, W, S), np.float32)
    for b in range(B):
        yb = res.results[b]["y"].reshape(HB, C, HH, W, S).transpose(1, 0, 2, 3, 4)
        out[b] = yb.reshape(C, H, W, S)
    return out, res


def kernel(x, w1, b1, w2, b2):
    out, _ = _run(x, w1, b1, w2, b2, trace=False)
    return out



# revision 20
# speedup vs baseline: 2.0590x; 2.0590x over previous
"""ConvAttention Trainium2 kernel (self-contained), v2.

Math: scores[b,h,w,t,s] = convQ(Q)[...,s] + convK(K)[...,t] + b2, softmax over t.
All t-independent terms (A_q, b2, and the conv-bias/boundary terms, which vary
only with (h,w)) cancel in the softmax, so
  attn[b,h,w,t] = softmax_t(conv5x5(w1k x)[b,h,w,t])
  out[b,c,h,w,s] = sum_t attn[t] (wv x_t + bv)   (indep of s; sum_t attn = 1)

Device pipeline per core (1 batch, bf16 data path):
  m = u2 @ x   (fused tap maps, 48 mm N=512) -> bf16 SBUF
  m --SBUF->SBUF DMA--> d2 [(dh%2, h) partitions, (dh//2, dw, w, t) free]
  conv: 30 accumulating mm (contraction 128 = both dh parities at once)
  softmax over t (no max-sub needed; logits are O(5))
  p --transpose DMA--> p2 [2, 24576] --broadcast DMA--> Pb [128, .]
  pw = x * Pb (DVE), out = sum_t wv2 @ pw_t (12 accum. mm / chunk), +bv drain
Output y [128=(hb,c), 2048=(h',w)] fp32; host broadcasts s and descrambles.
"""
import numpy as np
from contextlib import ExitStack

B, C, H, W, S = 8, 64, 64, 64, 12
K5 = 5
HB, HH = 2, 32
FREE = HH * W * S          # 24576 free cols per partition (h', w, t)
NT = 25                    # taps
WT = W * S                 # 768 cols per h'-row
NM = 48                    # m matmuls (N=512)
NMC = FREE // NM           # 512
NG = 8                     # transpose h'-groups (4 h'-rows each)
KD = 15                    # d2 free blocks (k=dh//2 in 0..2, dw in 0..4)
NCH = 4                    # stage-B chunks
CHX = FREE // NCH          # 6144 x-cols per chunk
CHW = CHX // S             # 512 hw-cols per chunk

_cache = {}


def _build_program(debug=False):
    import concourse.bass as bass
    import concourse.tile as tile
    from concourse import bacc, mybir
    f32 = mybir.dt.float32
    bf16 = mybir.dt.bfloat16

    nc = bacc.Bacc("TRN2", target_bir_lowering=False, debug=False, num_devices=8)
    x_in = nc.dram_tensor("x", [128, FREE], bf16, kind="ExternalInput")
    u2_in = nc.dram_tensor("u2", [128, 50], bf16, kind="ExternalInput")
    s5_in = nc.dram_tensor("s5", [64, 5 * 64], bf16, kind="ExternalInput")
    wv2_in = nc.dram_tensor("wv2", [128, 128], bf16, kind="ExternalInput")
    bv2_in = nc.dram_tensor("bv2", [128, 1], f32, kind="ExternalInput")
    y_out = nc.dram_tensor("y", [128, HH * W], f32, kind="ExternalOutput")
    if debug:
        dbg_m = nc.dram_tensor("dbg_m", [50, FREE], bf16,
                               kind="ExternalOutput")
        dbg_d = nc.dram_tensor("dbg_d", [64, NT * WT], bf16,
                               kind="ExternalOutput")
        dbg_p = nc.dram_tensor("dbg_p", [64, WT], bf16, kind="ExternalOutput")
        dbg_pb = nc.dram_tensor("dbg_pb", [128, CHX], bf16,
                                kind="ExternalOutput")

    with tile.TileContext(nc) as tc:
        with ExitStack() as ctx:
            cpool = ctx.enter_context(tc.tile_pool(name="consts", bufs=1))
            u2 = cpool.tile([128, 50], bf16)
            nc.sync.dma_start(u2[:], u2_in[:])
            s5 = cpool.tile([64, 5 * 64], bf16)
            nc.sync.dma_start(s5[:], s5_in[:])
            wv2 = cpool.tile([128, 128], bf16)
            nc.sync.dma_start(wv2[:], wv2_in[:])
            bv2 = cpool.tile([128, 1], f32)
            nc.sync.dma_start(bv2[:], bv2_in[:])

            xpool = ctx.enter_context(tc.tile_pool(name="x", bufs=1))
            x_sb = xpool.tile([128, FREE], bf16)
            for j in range(12):
                sl = bass.ts(j, FREE // 12)
                nc.sync.dma_start(x_sb[:, sl], x_in[:, sl])

            ppool = ctx.enter_context(tc.tile_pool(name="p", bufs=1))
            p_sb = ppool.tile([64, WT], bf16)
            ypool = ctx.enter_context(tc.tile_pool(name="y", bufs=1))
            y_sb = ypool.tile([128, HH * W], f32)
            drpool = ctx.enter_context(
                tc.tile_pool(name="dram", bufs=1, space="DRAM"))
            dram_m = drpool.tile([50, FREE], bf16)
            dram_p = drpool.tile([64, WT], bf16)

            # ---- stage A: tap maps -> transpose -> conv -> softmax ----
            with ExitStack() as actx:
                mpool = actx.enter_context(tc.tile_pool(name="m", bufs=1))
                m_sb = mpool.tile([50, FREE], bf16)
                psM = actx.enter_context(
                    tc.tile_pool(name="psM", bufs=4, space="PSUM"))
                for j in range(NM):
                    ps = psM.tile([50, NMC], f32)
                    nc.tensor.matmul(ps[:], u2[:], x_sb[:, bass.ts(j, NMC)],
                                     start=True, stop=True)
                    dst = m_sb[:, bass.ts(j, NMC)]
                    if j % 2 == 0:
                        nc.scalar.activation(
                            dst, ps[:], mybir.ActivationFunctionType.Identity)
                    else:
                        nc.vector.tensor_copy(dst, ps[:])
                    if j % 6 == 5:
                        sl = bass.ts(j // 6, 6 * NMC)
                        nc.sync.dma_start(dram_m[:, sl], m_sb[:, sl])

                dpool = actx.enter_context(tc.tile_pool(name="d", bufs=1))
                d_sb = dpool.tile([64, NT * WT], bf16)
                # dram_m rows (hb, r) cols (h', w, t)
                #   -> d partitions (hb, h'), free (r, w, t)
                for g in range(NG):
                    for hb in range(2):
                        src = dram_m[hb * NT:(hb + 1) * NT, :].rearrange(
                            "r (hp f) -> r hp f", hp=HH)[
                            :, g * 4:(g + 1) * 4, :].transpose([1, 0, 2])
                        dst = d_sb[hb * HH + g * 4: hb * HH + (g + 1) * 4, :]
                        nc.sync.dma_start(dst, src)

                if debug:
                    nc.sync.dma_start(dbg_m[:], m_sb[:])
                    nc.sync.dma_start(dbg_d[:], d_sb[:])

                psA = actx.enter_context(
                    tc.tile_pool(name="psA", bufs=2, space="PSUM"))
                smp = actx.enter_context(tc.tile_pool(name="smax", bufs=2))
                for wh in range(2):
                    a_ps = psA.tile([64, 384], f32)
                    order = [(dh, dw) for dh in range(5)
                             for dw in [2, 0, 1, 3, 4]]
                    for i, (dh, dw) in enumerate(order):
                        lo = max(wh * 32, 2 - dw)
                        hi = min(wh * 32 + 32, 66 - dw)
                        base = (dh * 5 + dw) * WT
                        rhs = d_sb[:, base + (lo + dw - 2) * S:
                                   base + (hi + dw - 2) * S]
                        out = a_ps[:, (lo - wh * 32) * S:(hi - wh * 32) * S]
                        nc.tensor.matmul(out, s5[:, bass.ts(dh, 64)], rhs,
                                         start=(i == 0), stop=(i == 24))
                    e_sb = smp.tile([64, 384], bf16)
                    nc.scalar.activation(e_sb[:], a_ps[:],
                                         mybir.ActivationFunctionType.Exp)
                    e3 = e_sb[:].rearrange("p (w t) -> p w t", t=S)
                    z = smp.tile([64, 32], f32)
                    nc.vector.tensor_reduce(z[:], e3, axis=mybir.AxisListType.X,
                                            op=mybir.AluOpType.add)
                    rcp = smp.tile([64, 32], f32)
                    nc.vector.reciprocal(rcp[:], z[:])
                    nc.vector.tensor_mul(
                        p_sb[:, bass.ts(wh, 384)].rearrange(
                            "p (w t) -> p w t", t=S),
                        e3, rcp[:].broadcast_to([64, 32, S]))

            nc.sync.dma_start(dram_p[:], p_sb[:])
            if debug:
                nc.sync.dma_start(dbg_p[:], p_sb[:])
            # flat view: dram_p [2=hb, (h', w, t)]
            p2v = dram_p[:].rearrange("(hb hp) f -> hb (hp f)", hb=2)

            # ---- stage B: broadcast p, weight x, t-fold matmul ----
            with ExitStack() as bctx:
                pbp = bctx.enter_context(tc.tile_pool(name="pb", bufs=2))
                pwp = bctx.enter_context(tc.tile_pool(name="pw", bufs=2))
                psO = bctx.enter_context(
                    tc.tile_pool(name="psO", bufs=2, space="PSUM"))
                for ch in range(NCH):
                    pb = pbp.tile([128, CHX], bf16)
                    # seed 8 consecutive rows per hb-block from DRAM, then 3
                    # doubling rounds over flat contiguous row ranges
                    for hb in range(2):
                        for sd in range(8):
                            nc.sync.dma_start(
                                pb[hb * 64 + sd: hb * 64 + sd + 1, :],
                                p2v[hb: hb + 1, bass.ts(ch, CHX)])
                        for rnd in range(3):
                            n = 8 << rnd
                            nc.sync.dma_start(
                                pb[hb * 64 + n: hb * 64 + 2 * n, :],
                                pb[hb * 64: hb * 64 + n, :])
                    if debug and ch == 0:
                        nc.sync.dma_start(dbg_pb[:], pb[:])
                    pw = pwp.tile([128, CHX], bf16)
                    nc.vector.tensor_mul(pw[:], x_sb[:, bass.ts(ch, CHX)],
                                         pb[:])
                    o_ps = psO.tile([128, CHW], f32)
                    pw3 = pw[:].rearrange("p (hw t) -> p hw t", t=S)
                    for t in range(S):
                        nc.tensor.matmul(o_ps[:], wv2[:], pw3[:, :, t:t + 1],
                                         start=(t == 0), stop=(t == S - 1))
                    nc.scalar.activation(
                        y_sb[:, bass.ts(ch, CHW)], o_ps[:],
                        mybir.ActivationFunctionType.Identity,
                        bias=bv2[:, 0:1])
            nc.sync.dma_start(y_out[:], y_sb[:])

    nc.compile()
    return nc


def _prep_weights(w1, b1, w2, b2):
    import ml_dtypes
    bf = ml_dtypes.bfloat16
    w1f = w1[:, :, 0, 0].astype(np.float32)
    wk, wv = w1f[64:128], w1f[128:192]
    bv = b1[128:192].astype(np.float32)
    w2k = w2[0, 64:128].astype(np.float32)                     # [c,5,5]
    u = np.tensordot(w2k, wk, axes=([0], [0])).reshape(NT, C)  # [25, 64]
    u2 = np.zeros((128, 50), np.float32)
    u2[0:64, 0:25] = u.T
    u2[64:128, 25:50] = u.T
    # s5[h_in, (dh, h_out)] = 1 iff h_in == h_out + dh - 2
    s5 = np.zeros((64, 5, 64), np.float32)
    for dh in range(5):
        for ho in range(64):
            hi = ho + dh - 2
            if 0 <= hi < 64:
                s5[hi, dh, ho] = 1.0
    s5 = s5.reshape(64, 5 * 64)
    wv2 = np.zeros((128, 128), np.float32)
    wv2[0:64, 0:64] = wv.T
    wv2[64:128, 64:128] = wv.T
    bv2 = np.concatenate([bv, bv]).reshape(128, 1).astype(np.float32)
    return (u2.astype(bf), s5.astype(bf), wv2.astype(bf), bv2)


def _run(x, w1, b1, w2, b2, trace=False):
    import ml_dtypes
    from concourse.bass_utils import run_bass_kernel_spmd
    bf = ml_dtypes.bfloat16
    if "nc" not in _cache:
        _cache["nc"] = _build_program()
    nc = _cache["nc"]
    u2, s5, wv2, bv2 = _prep_weights(w1, b1, w2, b2)
    in_maps = []
    for b in range(B):
        xb = np.ascontiguousarray(
            x[b].reshape(C, HB, HH, W * S).transpose(1, 0, 2, 3)
            .reshape(128, FREE)).astype(bf)
        in_maps.append({"x": xb, "u2": u2, "s5": s5, "wv2": wv2, "bv2": bv2})
    res = run_bass_kernel_spmd(nc, in_maps, core_ids=list(range(8)), trace=trace)
    out = np.empty((B, C, H, W, S), np.float32)
    for b in range(B):
        yb = res.results[b]["y"].reshape(HB, C, HH, W)  # [(hb,c), h', w]
        out[b] = yb.transpose(1, 0, 2, 3).reshape(C, H, W)[..., None]
    return out, res


def kernel(x, w1, b1, w2, b2):
    out, _ = _run(x, w1, b1, w2, b2, trace=False)
    return out


# revision 22
# speedup vs baseline: 2.2115x; 1.0740x over previous
"""ConvAttention Trainium2 kernel (self-contained), v3.

Math: scores[b,h,w,t,s] = convQ(Q)[...,s] + convK(K)[...,t] + b2, softmax over t.
All t-independent terms (A_q, b2, conv bias, boundary terms) cancel in the
softmax, so
  attn[b,h,w,t] = softmax_t(conv5x5(w1k x)[b,h,w,t])
  out[b,c,h,w,s] = sum_t attn[t] (wv x_t + bv)   (indep of s; sum_t attn = 1)

Device pipeline per core (1 batch, bf16 data path):
  m = u2 @ x   (fused tap maps, 48 mm N=512; PSUM evac split ACT/DVE)
  m --50 SBUF->SBUF fanout DMAs--> d2 [(dh%2, h) part, (dh//2, dw, w, t) free]
  conv: 30 accumulating mm, contraction 128 (both dh parities), 3 lhsT loads
  softmax over t (logits are O(4); no max-subtraction needed)
  p -> dram_p -> Pb [128, FREE] (16 seed reads + 6 flat doubling DMAs)
  pw = x * Pb (DVE, 4 chunks), out = sum_t wv2 @ pw_t (12 mm/chunk), +bv drain
Output y [128=(hb,c), 2048=(h',w)] fp32; host broadcasts s and descrambles.
DMAs are spread across both HWDGE rings (nc.sync / nc.scalar).
"""
import numpy as np
from contextlib import ExitStack

B, C, H, W, S = 8, 64, 64, 64, 12
K5 = 5
HB, HH = 2, 32
FREE = HH * W * S          # 24576 free cols per partition (h', w, t)
NT = 25                    # taps
WT = W * S                 # 768 cols per h'-row
NM = 48                    # m matmuls (N=512)
NMC = FREE // NM           # 512
KD = 15                    # d2 free blocks (k=dh//2 in 0..2, dw in 0..4)
NCH = 4                    # stage-B chunks
CHX = FREE // NCH          # 6144 x-cols per chunk
CHW = CHX // S             # 512 hw-cols per chunk

_cache = {}


def _build_program(debug=False):
    import concourse.bass as bass
    import concourse.tile as tile
    from concourse import bacc, mybir
    f32 = mybir.dt.float32
    bf16 = mybir.dt.bfloat16

    nc = bacc.Bacc("TRN2", target_bir_lowering=False, debug=False, num_devices=8)
    x_in = nc.dram_tensor("x", [128, FREE], bf16, kind="ExternalInput")
    u2_in = nc.dram_tensor("u2", [128, 50], bf16, kind="ExternalInput")
    s5_in = nc.dram_tensor("s5", [128, 3 * 64], bf16, kind="ExternalInput")
    wv2_in = nc.dram_tensor("wv2", [128, 128], bf16, kind="ExternalInput")
    bv2_in = nc.dram_tensor("bv2", [128, 1], f32, kind="ExternalInput")
    y_out = nc.dram_tensor("y", [128, HH * W], f32, kind="ExternalOutput")
    if debug:
        dbg_d = nc.dram_tensor("dbg_d", [128, KD * WT], bf16,
                               kind="ExternalOutput")
        dbg_p = nc.dram_tensor("dbg_p", [64, WT], bf16, kind="ExternalOutput")
        dbg_pb = nc.dram_tensor("dbg_pb", [128, FREE], bf16,
                                kind="ExternalOutput")

    def ring(i):
        return nc.sync if i % 2 == 0 else nc.scalar

    with tile.TileContext(nc) as tc:
        with ExitStack() as ctx:
            cpool = ctx.enter_context(tc.tile_pool(name="consts", bufs=1))
            u2 = cpool.tile([128, 50], bf16)
            nc.sync.dma_start(u2[:], u2_in[:])
            s5 = cpool.tile([128, 3 * 64], bf16)
            nc.scalar.dma_start(s5[:], s5_in[:])
            wv2 = cpool.tile([128, 128], bf16)
            nc.sync.dma_start(wv2[:], wv2_in[:])
            bv2 = cpool.tile([128, 1], f32)
            nc.scalar.dma_start(bv2[:], bv2_in[:])

            xpool = ctx.enter_context(tc.tile_pool(name="x", bufs=1))
            x_sb = xpool.tile([128, FREE], bf16)
            for j in range(12):
                sl = bass.ts(j, FREE // 12)
                ring(j).dma_start(x_sb[:, sl], x_in[:, sl])

            ppool = ctx.enter_context(tc.tile_pool(name="p", bufs=1))
            p_sb = ppool.tile([64, WT], bf16)
            ypool = ctx.enter_context(tc.tile_pool(name="y", bufs=1))
            y_sb = ypool.tile([128, HH * W], f32)
            drpool = ctx.enter_context(
                tc.tile_pool(name="dram", bufs=1, space="DRAM"))
            dram_p = drpool.tile([64, WT], bf16)

            # ---- stage A: tap maps -> fanout transpose -> conv -> softmax ----
            with ExitStack() as actx:
                mpool = actx.enter_context(tc.tile_pool(name="m", bufs=1))
                m_sb = mpool.tile([50, FREE], bf16)
                psM = actx.enter_context(
                    tc.tile_pool(name="psM", bufs=4, space="PSUM"))
                for j in range(NM):
                    ps = psM.tile([50, NMC], f32)
                    nc.tensor.matmul(ps[:], u2[:], x_sb[:, bass.ts(j, NMC)],
                                     start=True, stop=True)
                    dst = m_sb[:, bass.ts(j, NMC)]
                    if j % 2 == 0:
                        nc.scalar.activation(
                            dst, ps[:], mybir.ActivationFunctionType.Identity)
                    else:
                        nc.vector.tensor_copy(dst, ps[:])

                dpool = actx.enter_context(tc.tile_pool(name="d2", bufs=1))
                d2 = dpool.tile([128, KD * WT], bf16)
                # zero the unused (dhp=1, k=2) block
                nc.vector.memset(d2[64:128, 10 * WT:15 * WT], 0.0)
                # m row (hb, dh, dw) [.., (h', w, t)]
                #  -> d2 partitions (dh%2)*64 + hb*32 + h', free (dh//2, dw, w, t)
                for row in range(50):
                    hb, r = row // NT, row % NT
                    dh, dw = r // 5, r % 5
                    dhp, k = dh % 2, dh // 2
                    src = m_sb[row:row + 1, :].rearrange(
                        "p (hp f) -> p hp f", hp=HH)
                    dst = d2[dhp * 64 + hb * HH: dhp * 64 + (hb + 1) * HH,
                             (k * 5 + dw) * WT:(k * 5 + dw + 1) * WT]
                    ring(row).dma_start(dst, src)
                if debug:
                    nc.sync.dma_start(dbg_d[:], d2[:])

                psA = actx.enter_context(
                    tc.tile_pool(name="psA", bufs=2, space="PSUM"))
                smp = actx.enter_context(tc.tile_pool(name="smax", bufs=2))
                for wh in range(2):
                    a_ps = psA.tile([64, 384], f32)
                    # (k=0, dw=2) first: full w-range, resets the PSUM bank
                    order = [(k, dw) for k in range(3)
                             for dw in [2, 0, 1, 3, 4]]
                    for i, (k, dw) in enumerate(order):
                        lo = max(wh * 32, 2 - dw)
                        hi = min(wh * 32 + 32, 66 - dw)
                        base = (k * 5 + dw) * WT
                        rhs = d2[:, base + (lo + dw - 2) * S:
                                 base + (hi + dw - 2) * S]
                        out = a_ps[:, (lo - wh * 32) * S:(hi - wh * 32) * S]
                        nc.tensor.matmul(out, s5[:, bass.ts(k, 64)], rhs,
                                         start=(i == 0), stop=(i == 14))
                    e_sb = smp.tile([64, 384], bf16)
                    nc.scalar.activation(e_sb[:], a_ps[:],
                                         mybir.ActivationFunctionType.Exp)
                    e3 = e_sb[:].rearrange("p (w t) -> p w t", t=S)
                    z = smp.tile([64, 32], f32)
                    nc.vector.tensor_reduce(z[:], e3, axis=mybir.AxisListType.X,
                                            op=mybir.AluOpType.add)
                    rcp = smp.tile([64, 32], f32)
                    nc.vector.reciprocal(rcp[:], z[:])
                    nc.vector.tensor_mul(
                        p_sb[:, bass.ts(wh, 384)].rearrange(
                            "p (w t) -> p w t", t=S),
                        e3, rcp[:].broadcast_to([64, 32, S]))
            if debug:
                nc.sync.dma_start(dbg_p[:], p_sb[:])

            nc.sync.dma_start(dram_p[:], p_sb[:])
            # flat view: dram_p [2=hb, (h', w, t)]
            p2v = dram_p[:].rearrange("(hb hp) f -> hb (hp f)", hb=2)

            # ---- stage B: full-size broadcast, then chunked mult + t-fold ----
            with ExitStack() as bctx:
                pbp = bctx.enter_context(tc.tile_pool(name="pb", bufs=1))
                pwp = bctx.enter_context(tc.tile_pool(name="pw", bufs=2))
                psO = bctx.enter_context(
                    tc.tile_pool(name="psO", bufs=2, space="PSUM"))
                pb = pbp.tile([128, FREE], bf16)
                for hb in range(2):
                    for sd in range(8):
                        ring(sd).dma_start(
                            pb[hb * 64 + sd: hb * 64 + sd + 1, :],
                            p2v[hb: hb + 1, :])
                    for rnd in range(3):
                        n = 8 << rnd
                        ring(hb).dma_start(
                            pb[hb * 64 + n: hb * 64 + 2 * n, :],
                            pb[hb * 64: hb * 64 + n, :])
                if debug:
                    nc.sync.dma_start(dbg_pb[:], pb[:])
                for ch in range(NCH):
                    pw = pwp.tile([128, CHX], bf16)
                    nc.vector.tensor_mul(pw[:], x_sb[:, bass.ts(ch, CHX)],
                                         pb[:, bass.ts(ch, CHX)])
                    o_ps = psO.tile([128, CHW], f32)
                    pw3 = pw[:].rearrange("p (hw t) -> p hw t", t=S)
                    for t in range(S):
                        nc.tensor.matmul(o_ps[:], wv2[:], pw3[:, :, t:t + 1],
                                         start=(t == 0), stop=(t == S - 1))
                    nc.scalar.activation(
                        y_sb[:, bass.ts(ch, CHW)], o_ps[:],
                        mybir.ActivationFunctionType.Identity,
                        bias=bv2[:, 0:1])
            nc.sync.dma_start(y_out[:], y_sb[:])

    nc.compile()
    return nc


def _prep_weights(w1, b1, w2, b2):
    import ml_dtypes
    bf = ml_dtypes.bfloat16
    w1f = w1[:, :, 0, 0].astype(np.float32)
    wk, wv = w1f[64:128], w1f[128:192]
    bv = b1[128:192].astype(np.float32)
    w2k = w2[0, 64:128].astype(np.float32)                     # [c,5,5]
    u = np.tensordot(w2k, wk, axes=([0], [0])).reshape(NT, C)  # [25, 64]
    u2 = np.zeros((128, 50), np.float32)
    u2[0:64, 0:25] = u.T
    u2[64:128, 25:50] = u.T
    # s5[(dhp, h_in), (k, h_out)] = 1 iff h_in == h_out + (2k+dhp) - 2
    s5 = np.zeros((2, 64, 3, 64), np.float32)
    for dh in range(5):
        k, dhp = dh // 2, dh % 2
        for ho in range(64):
            hi = ho + dh - 2
            if 0 <= hi < 64:
                s5[dhp, hi, k, ho] = 1.0
    s5 = s5.reshape(128, 3 * 64)
    wv2 = np.zeros((128, 128), np.float32)
    wv2[0:64, 0:64] = wv.T
    wv2[64:128, 64:128] = wv.T
    bv2 = np.concatenate([bv, bv]).reshape(128, 1).astype(np.float32)
    return (u2.astype(bf), s5.astype(bf), wv2.astype(bf), bv2)


def _run(x, w1, b1, w2, b2, trace=False):
    import ml_dtypes
    from concourse.bass_utils import run_bass_kernel_spmd
    bf = ml_dtypes.bfloat16
    if "nc" not in _cache:
        _cache["nc"] = _build_program()
    nc = _cache["nc"]
    u2, s5, wv2, bv2 = _prep_weights(w1, b1, w2, b2)
    in_maps = []
    for b in range(B):
        xb = np.ascontiguousarray(
            x[b].reshape(C, HB, HH, W * S).transpose(1, 0, 2, 3)
            .reshape(128, FREE)).astype(bf)
        in_maps.append({"x": xb, "u2": u2, "s5": s5, "wv2": wv2, "bv2": bv2})
    res = run_bass_kernel_spmd(nc, in_maps, core_ids=list(range(8)), trace=trace)
    out = np.empty((B, C, H, W, S), np.float32)
    for b in range(B):
        yb = res.results[b]["y"].reshape(HB, C, HH, W)  # [(hb,c), h', w]
        out[b] = yb.transpose(1, 0, 2, 3).reshape(C, H, W)[..., None]
    return out, res


def kernel(x, w1, b1, w2, b2):
    out, _ = _run(x, w1, b1, w2, b2, trace=False)
    return out


# revision 23
# speedup vs baseline: 2.3683x; 1.0709x over previous
"""ConvAttention Trainium2 kernel (self-contained), v4.

Math: scores[b,h,w,t,s] = convQ(Q)[...,s] + convK(K)[...,t] + b2, softmax over t.
All t-independent terms (A_q, b2, conv bias, boundary terms) cancel in the
softmax, so
  attn[b,h,w,t] = softmax_t(conv5x5(w1k x)[b,h,w,t])
  out[b,c,h,w,s] = sum_t attn[t] (wv x_t + bv)   (indep of s; sum_t attn = 1)

Device pipeline per core (1 batch, bf16 data path):
  m = u2 @ x  (48 mm N=512; PSUM evac split ACT/DVE; chunked writes to DRAM)
  dram_m --50 per-row fanout reads--> d2 [(dh%2,h) part, (dh//2,dw,w,t) free]
  conv: 30 accumulating mm, contraction 128, 3 lhsT loads
  softmax over t (logits O(4): no max-subtraction)
  p -> dram_p -> p2_sb [2, FREE]; pb = sel2.T @ p2 (48 bcast mm + evac)
  pw = x * pb written t-major (DVE), out = sum_t wv2 @ pw_t (dense rhs mm)
Output y [128=(hb,c), 2048=(h',w)] fp32; host broadcasts s and descrambles.
DMAs alternate between the two HWDGE rings (nc.sync / nc.scalar).
"""
import numpy as np
from contextlib import ExitStack

B, C, H, W, S = 8, 64, 64, 64, 12
K5 = 5
HB, HH = 2, 32
FREE = HH * W * S          # 24576 free cols per partition (h', w, t)
NT = 25                    # taps
WT = W * S                 # 768 cols per h'-row
NM = 48                    # m matmuls (N=512)
NMC = FREE // NM           # 512
KD = 15                    # d2 free blocks (k=dh//2 in 0..2, dw in 0..4)
NCH = 4                    # stage-B chunks
CHX = FREE // NCH          # 6144 x-cols per chunk
CHW = CHX // S             # 512 hw-cols per chunk

_cache = {}


def _build_program(debug=False):
    import concourse.bass as bass
    import concourse.tile as tile
    from concourse import bacc, mybir
    f32 = mybir.dt.float32
    bf16 = mybir.dt.bfloat16

    nc = bacc.Bacc("TRN2", target_bir_lowering=False, debug=False, num_devices=8)
    x_in = nc.dram_tensor("x", [128, FREE], bf16, kind="ExternalInput")
    u2_in = nc.dram_tensor("u2", [128, 50], bf16, kind="ExternalInput")
    s5_in = nc.dram_tensor("s5", [128, 3 * 64], bf16, kind="ExternalInput")
    sel2_in = nc.dram_tensor("sel2", [2, 128], bf16, kind="ExternalInput")
    wv2_in = nc.dram_tensor("wv2", [128, 128], bf16, kind="ExternalInput")
    bv2_in = nc.dram_tensor("bv2", [128, 1], f32, kind="ExternalInput")
    y_out = nc.dram_tensor("y", [128, HH * W], f32, kind="ExternalOutput")
    if debug:
        dbg_d = nc.dram_tensor("dbg_d", [128, KD * WT], bf16,
                               kind="ExternalOutput")
        dbg_p = nc.dram_tensor("dbg_p", [64, WT], bf16, kind="ExternalOutput")
        dbg_pb = nc.dram_tensor("dbg_pb", [128, FREE], bf16,
                                kind="ExternalOutput")

    def ring(i):
        return nc.sync if i % 2 == 0 else nc.scalar

    with tile.TileContext(nc) as tc:
        with ExitStack() as ctx:
            cpool = ctx.enter_context(tc.tile_pool(name="consts", bufs=1))
            u2 = cpool.tile([128, 50], bf16)
            nc.sync.dma_start(u2[:], u2_in[:])
            s5 = cpool.tile([128, 3 * 64], bf16)
            nc.scalar.dma_start(s5[:], s5_in[:])
            sel2 = cpool.tile([2, 128], bf16)
            nc.sync.dma_start(sel2[:], sel2_in[:])
            wv2 = cpool.tile([128, 128], bf16)
            nc.scalar.dma_start(wv2[:], wv2_in[:])
            bv2 = cpool.tile([128, 1], f32)
            nc.sync.dma_start(bv2[:], bv2_in[:])

            xpool = ctx.enter_context(tc.tile_pool(name="x", bufs=1))
            x_sb = xpool.tile([128, FREE], bf16)
            for j in range(12):
                sl = bass.ts(j, FREE // 12)
                ring(j).dma_start(x_sb[:, sl], x_in[:, sl])

            ppool = ctx.enter_context(tc.tile_pool(name="p", bufs=1))
            p_sb = ppool.tile([64, WT], bf16)
            ypool = ctx.enter_context(tc.tile_pool(name="y", bufs=1))
            y_sb = ypool.tile([128, HH * W], f32)
            drpool = ctx.enter_context(
                tc.tile_pool(name="dram", bufs=1, space="DRAM"))
            dram_m = drpool.tile([50, FREE], bf16)
            dram_p = drpool.tile([64, WT], bf16)

            # ---- stage A: tap maps -> fanout transpose -> conv -> softmax ----
            with ExitStack() as actx:
                mpool = actx.enter_context(tc.tile_pool(name="m", bufs=1))
                m_sb = mpool.tile([50, FREE], bf16)
                psM = actx.enter_context(
                    tc.tile_pool(name="psM", bufs=4, space="PSUM"))
                for j in range(NM):
                    ps = psM.tile([50, NMC], f32)
                    nc.tensor.matmul(ps[:], u2[:], x_sb[:, bass.ts(j, NMC)],
                                     start=True, stop=True)
                    dst = m_sb[:, bass.ts(j, NMC)]
                    if j % 2 == 0:
                        nc.scalar.activation(
                            dst, ps[:], mybir.ActivationFunctionType.Identity)
                    else:
                        nc.vector.tensor_copy(dst, ps[:])
                    if j % 6 == 5:
                        sl = bass.ts(j // 6, 6 * NMC)
                        ring(j // 6).dma_start(dram_m[:, sl], m_sb[:, sl])

                dpool = actx.enter_context(tc.tile_pool(name="d2", bufs=1))
                d2 = dpool.tile([128, KD * WT], bf16)
                nc.vector.memset(d2[64:128, 10 * WT:15 * WT], 0.0)
                # dram_m row (hb, dh, dw) [(h', w, t)]
                #  -> d2 partitions (dh%2)*64 + hb*32 + h',
                #     free (dh//2, dw, w, t)
                for row in range(50):
                    hb, r = row // NT, row % NT
                    dh, dw = r // 5, r % 5
                    dhp, k = dh % 2, dh // 2
                    src = dram_m[row:row + 1, :].rearrange(
                        "p (hp f) -> p hp f", hp=HH)
                    dst = d2[dhp * 64 + hb * HH: dhp * 64 + (hb + 1) * HH,
                             (k * 5 + dw) * WT:(k * 5 + dw + 1) * WT]
                    ring(row).dma_start(dst, src)
                if debug:
                    nc.sync.dma_start(dbg_d[:], d2[:])

                psA = actx.enter_context(
                    tc.tile_pool(name="psA", bufs=2, space="PSUM"))
                smp = actx.enter_context(tc.tile_pool(name="smax", bufs=2))
                for wh in range(2):
                    a_ps = psA.tile([64, 384], f32)
                    # (k=0, dw=2) first: full w-range, resets the PSUM bank
                    order = [(k, dw) for k in range(3)
                             for dw in [2, 0, 1, 3, 4]]
                    for i, (k, dw) in enumerate(order):
                        lo = max(wh * 32, 2 - dw)
                        hi = min(wh * 32 + 32, 66 - dw)
                        base = (k * 5 + dw) * WT
                        rhs = d2[:, base + (lo + dw - 2) * S:
                                 base + (hi + dw - 2) * S]
                        out = a_ps[:, (lo - wh * 32) * S:(hi - wh * 32) * S]
                        nc.tensor.matmul(out, s5[:, bass.ts(k, 64)], rhs,
                                         start=(i == 0), stop=(i == 14))
                    e_sb = smp.tile([64, 384], bf16)
                    nc.scalar.activation(e_sb[:], a_ps[:],
                                         mybir.ActivationFunctionType.Exp)
                    e3 = e_sb[:].rearrange("p (w t) -> p w t", t=S)
                    z = smp.tile([64, 32], f32)
                    nc.vector.tensor_reduce(z[:], e3, axis=mybir.AxisListType.X,
                                            op=mybir.AluOpType.add)
                    rcp = smp.tile([64, 32], f32)
                    nc.vector.reciprocal(rcp[:], z[:])
                    nc.vector.tensor_mul(
                        p_sb[:, bass.ts(wh, 384)].rearrange(
                            "p (w t) -> p w t", t=S),
                        e3, rcp[:].broadcast_to([64, 32, S]))
            if debug:
                nc.sync.dma_start(dbg_p[:], p_sb[:])

            nc.sync.dma_start(dram_p[:], p_sb[:])

            # ---- stage B: PE broadcast of p, chunked mult + dense t-fold ----
            with ExitStack() as bctx:
                p2pool = bctx.enter_context(tc.tile_pool(name="p2", bufs=1))
                p2_sb = p2pool.tile([2, FREE], bf16)
                nc.scalar.dma_start(
                    p2_sb[:], dram_p[:].rearrange("(hb hp) f -> hb (hp f)",
                                                  hb=2))
                pbp = bctx.enter_context(tc.tile_pool(name="pb", bufs=1))
                pb = pbp.tile([128, FREE], bf16)
                psB = bctx.enter_context(
                    tc.tile_pool(name="psB", bufs=4, space="PSUM"))
                for j in range(NM):
                    bps = psB.tile([128, NMC], f32)
                    nc.tensor.matmul(bps[:], sel2[:],
                                     p2_sb[:, bass.ts(j, NMC)],
                                     start=True, stop=True)
                    dst = pb[:, bass.ts(j, NMC)]
                    if j % 2 == 0:
                        nc.scalar.activation(
                            dst, bps[:], mybir.ActivationFunctionType.Identity)
                    else:
                        nc.vector.tensor_copy(dst, bps[:])
                if debug:
                    nc.sync.dma_start(dbg_pb[:], pb[:])

                pwp = bctx.enter_context(tc.tile_pool(name="pw", bufs=2))
                psO = bctx.enter_context(
                    tc.tile_pool(name="psO", bufs=2, space="PSUM"))
                for ch in range(NCH):
                    pw = pwp.tile([128, CHX], bf16)  # t-major: (t, hw)
                    pwv = pw[:].rearrange("p (t hw) -> p hw t", t=S)
                    x3 = x_sb[:, bass.ts(ch, CHX)].rearrange(
                        "p (hw t) -> p hw t", t=S)
                    pb3 = pb[:, bass.ts(ch, CHX)].rearrange(
                        "p (hw t) -> p hw t", t=S)
                    nc.vector.tensor_mul(pwv, x3, pb3)
                    o_ps = psO.tile([128, CHW], f32)
                    for t in range(S):
                        nc.tensor.matmul(o_ps[:], wv2[:],
                                         pw[:, bass.ts(t, CHW)],
                                         start=(t == 0), stop=(t == S - 1))
                    nc.scalar.activation(
                        y_sb[:, bass.ts(ch, CHW)], o_ps[:],
                        mybir.ActivationFunctionType.Identity,
                        bias=bv2[:, 0:1])
            nc.sync.dma_start(y_out[:], y_sb[:])

    nc.compile()
    return nc


def _prep_weights(w1, b1, w2, b2):
    import ml_dtypes
    bf = ml_dtypes.bfloat16
    w1f = w1[:, :, 0, 0].astype(np.float32)
    wk, wv = w1f[64:128], w1f[128:192]
    bv = b1[128:192].astype(np.float32)
    w2k = w2[0, 64:128].astype(np.float32)                     # [c,5,5]
    u = np.tensordot(w2k, wk, axes=([0], [0])).reshape(NT, C)  # [25, 64]
    u2 = np.zeros((128, 50), np.float32)
    u2[0:64, 0:25] = u.T
    u2[64:128, 25:50] = u.T
    # s5[(dhp, h_in), (k, h_out)] = 1 iff h_in == h_out + (2k+dhp) - 2
    s5 = np.zeros((2, 64, 3, 64), np.float32)
    for dh in range(5):
        k, dhp = dh // 2, dh % 2
        for ho in range(64):
            hi = ho + dh - 2
            if 0 <= hi < 64:
                s5[dhp, hi, k, ho] = 1.0
    s5 = s5.reshape(128, 3 * 64)
    sel2 = np.zeros((2, 128), np.float32)
    sel2[0, 0:64] = 1.0
    sel2[1, 64:128] = 1.0
    wv2 = np.zeros((128, 128), np.float32)
    wv2[0:64, 0:64] = wv.T
    wv2[64:128, 64:128] = wv.T
    bv2 = np.concatenate([bv, bv]).reshape(128, 1).astype(np.float32)
    return (u2.astype(bf), s5.astype(bf), sel2.astype(bf), wv2.astype(bf),
            bv2)


def _run(x, w1, b1, w2, b2, trace=False):
    import ml_dtypes
    from concourse.bass_utils import run_bass_kernel_spmd
    bf = ml_dtypes.bfloat16
    if "nc" not in _cache:
        _cache["nc"] = _build_program()
    nc = _cache["nc"]
    u2, s5, sel2, wv2, bv2 = _prep_weights(w1, b1, w2, b2)
    in_maps = []
    for b in range(B):
        xb = np.ascontiguousarray(
            x[b].reshape(C, HB, HH, W * S).transpose(1, 0, 2, 3)
            .reshape(128, FREE)).astype(bf)
        in_maps.append({"x": xb, "u2": u2, "s5": s5, "sel2": sel2,
                       "wv2": wv2, "bv2": bv2})
    res = run_bass_kernel_spmd(nc, in_maps, core_ids=list(range(8)), trace=trace)
    out = np.empty((B, C, H, W, S), np.float32)
    for b in range(B):
        yb = res.results[b]["y"].reshape(HB, C, HH, W)  # [(hb,c), h', w]
        out[b] = yb.transpose(1, 0, 2, 3).reshape(C, H, W)[..., None]
    return out, res


def kernel(x, w1, b1, w2, b2):
    out, _ = _run(x, w1, b1, w2, b2, trace=False)
    return out


# revision 29
# speedup vs baseline: 3.8160x; 1.6113x over previous
"""ConvAttention Trainium2 kernel (self-contained), v5.

Math: scores[b,h,w,t,s] = convQ(Q)[...,s] + convK(K)[...,t] + b2, softmax over t.
All t-independent terms (A_q, b2, conv bias, boundary terms) cancel in the
softmax, so
  attn[b,h,w,t] = softmax_t(conv5x5(w1k x)[b,h,w,t])
  out[b,c,h,w,s] = sum_t attn[t] (wv x_t + bv)   (indep of s; sum_t attn = 1)

Device pipeline per core (1 batch, bf16 data path):
  stage A (c-major x):  m = u2 @ x (48 mm) -> dram_m -> 50 fanout reads
    -> d2 [(dh%2,h) part, (dh//2,dw,w,t) free] -> conv (30 mm, K=128)
    -> softmax over t -> p [64=(hb,h'), (w,t)] -> dram_p
  stage B (hw-major x2 [128=(h,wb), (w',c,t)]):  p_hw [128,(w',t)] aligns
    with partitions - NO broadcast. pw = x2 * p_hw (c-axis stride-0 bcast),
    q = sum_t pw (dense DVE reduce), 16 PE transposes of q (w'-pair, c)
    tiles, y = wvp @ qT (16 mm N=128), +bv drain.
Output y [128=(w'a,c), 2048=(jj,h,wb)] fp32; host broadcasts s + descrambles.
"""
import numpy as np
from contextlib import ExitStack

B, C, H, W, S = 8, 64, 64, 64, 12
K5 = 5
HB, HH = 2, 32
FREE = HH * W * S          # 24576 free cols per partition
NT = 25                    # taps
WT = W * S                 # 768
NM = 48                    # m matmuls (N=512)
NMC = FREE // NM           # 512
KD = 15                    # d2 free blocks
WP = 32                    # w' per wb block
NCH = 4                    # stage-B chunks (8 w' each)
CHX = FREE // NCH          # 6144 cols (8 w' x 64 c x 12 t)
NTL = 16                   # transpose tiles (w'-pairs)

_cache = {}


def _build_program(debug=False):
    import concourse.bass as bass
    import concourse.tile as tile
    from concourse import bacc, mybir
    f32 = mybir.dt.float32
    bf16 = mybir.dt.bfloat16

    nc = bacc.Bacc("TRN2", target_bir_lowering=False, debug=False, num_devices=8)
    x_in = nc.dram_tensor("x", [128, FREE], bf16, kind="ExternalInput")
    x2_in = nc.dram_tensor("x2", [128, FREE], bf16, kind="ExternalInput")
    u2_in = nc.dram_tensor("u2", [128, 50], bf16, kind="ExternalInput")
    s5_in = nc.dram_tensor("s5", [128, 3 * 64], bf16, kind="ExternalInput")
    id_in = nc.dram_tensor("ident", [128, 128], f32, kind="ExternalInput")
    wvp_in = nc.dram_tensor("wvp", [128, 128], bf16, kind="ExternalInput")
    bvp_in = nc.dram_tensor("bvp", [128, 1], f32, kind="ExternalInput")
    y_out = nc.dram_tensor("y", [128, NTL * 128], f32, kind="ExternalOutput")
    if debug:
        dbg_p = nc.dram_tensor("dbg_p", [64, WT], bf16, kind="ExternalOutput")
        dbg_ph = nc.dram_tensor("dbg_ph", [128, WP * S], bf16,
                                kind="ExternalOutput")
        dbg_q = nc.dram_tensor("dbg_q", [128, NTL * 128], f32,
                               kind="ExternalOutput")

    def ring(i):
        return nc.sync if i % 2 == 0 else nc.scalar

    with tile.TileContext(nc) as tc:
        with ExitStack() as ctx:
            cpool = ctx.enter_context(tc.tile_pool(name="consts", bufs=1))
            u2 = cpool.tile([128, 50], bf16)
            nc.sync.dma_start(u2[:], u2_in[:])
            s5 = cpool.tile([128, 3 * 64], bf16)
            nc.scalar.dma_start(s5[:], s5_in[:])
            ident = cpool.tile([128, 128], f32)
            nc.sync.dma_start(ident[:], id_in[:])
            wvp = cpool.tile([128, 128], bf16)
            nc.scalar.dma_start(wvp[:], wvp_in[:])
            bvp = cpool.tile([128, 1], f32)
            nc.sync.dma_start(bvp[:], bvp_in[:])

            xpool = ctx.enter_context(tc.tile_pool(name="x", bufs=1))
            x_sb = xpool.tile([128, FREE], bf16)
            x2_sb = xpool.tile([128, FREE], bf16)
            for j in range(12):
                sl = bass.ts(j, FREE // 12)
                ring(j).dma_start(x_sb[:, sl], x_in[:, sl])
            for j in range(12):
                sl = bass.ts(j, FREE // 12)
                ring(j).dma_start(x2_sb[:, sl], x2_in[:, sl])

            ppool = ctx.enter_context(tc.tile_pool(name="p", bufs=1))
            p_sb = ppool.tile([64, WT], bf16)
            p_hw = ppool.tile([128, WP * S], bf16)
            ypool = ctx.enter_context(tc.tile_pool(name="y", bufs=1))
            y_sb = ypool.tile([128, NTL * 128], f32)
            q_sb = ypool.tile([128, NTL * 128], f32)
            drpool = ctx.enter_context(
                tc.tile_pool(name="dram", bufs=1, space="DRAM"))
            dram_m = drpool.tile([50, FREE], bf16)
            dram_p = drpool.tile([64, WT], bf16)

            # ---- stage A: tap maps -> fanout transpose -> conv -> softmax ----
            with ExitStack() as actx:
                mpool = actx.enter_context(tc.tile_pool(name="m", bufs=1))
                m_sb = mpool.tile([50, FREE], bf16)
                psM = actx.enter_context(
                    tc.tile_pool(name="psM", bufs=4, space="PSUM"))
                for j in range(NM):
                    ps = psM.tile([50, NMC], f32)
                    nc.tensor.matmul(ps[:], u2[:], x_sb[:, bass.ts(j, NMC)],
                                     start=True, stop=True)
                    dst = m_sb[:, bass.ts(j, NMC)]
                    if j % 2 == 0:
                        nc.scalar.activation(
                            dst, ps[:], mybir.ActivationFunctionType.Identity)
                    else:
                        nc.vector.tensor_copy(dst, ps[:])
                    if j % 6 == 5:
                        sl = bass.ts(j // 6, 6 * NMC)
                        ring(j // 6).dma_start(dram_m[:, sl], m_sb[:, sl])

                dpool = actx.enter_context(tc.tile_pool(name="d2", bufs=1))
                d2 = dpool.tile([128, KD * WT], bf16)
                nc.vector.memset(d2[64:128, 10 * WT:15 * WT], 0.0)
                # dram_m row (hb, dh, dw) [(h', w, t)]
                #  -> d2 partitions (dh%2)*64 + hb*32 + h', free (dh//2,dw,w,t)
                for row in range(50):
                    hb, r = row // NT, row % NT
                    dh, dw = r // 5, r % 5
                    dhp, k = dh % 2, dh // 2
                    src = dram_m[row:row + 1, :].rearrange(
                        "p (hp f) -> p hp f", hp=HH)
                    dst = d2[dhp * 64 + hb * HH: dhp * 64 + (hb + 1) * HH,
                             (k * 5 + dw) * WT:(k * 5 + dw + 1) * WT]
                    ring(row).dma_start(dst, src)

                psA = actx.enter_context(
                    tc.tile_pool(name="psA", bufs=2, space="PSUM"))
                smp = actx.enter_context(tc.tile_pool(name="smax", bufs=2))
                for wh in range(2):
                    a_ps = psA.tile([64, 384], f32)
                    # (k=0, dw=2) first: full w-range, resets the PSUM bank
                    order = [(k, dw) for k in range(3)
                             for dw in [2, 0, 1, 3, 4]]
                    for i, (k, dw) in enumerate(order):
                        lo = max(wh * 32, 2 - dw)
                        hi = min(wh * 32 + 32, 66 - dw)
                        base = (k * 5 + dw) * WT
                        rhs = d2[:, base + (lo + dw - 2) * S:
                                 base + (hi + dw - 2) * S]
                        out = a_ps[:, (lo - wh * 32) * S:(hi - wh * 32) * S]
                        nc.tensor.matmul(out, s5[:, bass.ts(k, 64)], rhs,
                                         start=(i == 0), stop=(i == 14))
                    e_sb = smp.tile([64, 384], bf16)
                    nc.scalar.activation(e_sb[:], a_ps[:],
                                         mybir.ActivationFunctionType.Exp)
                    e3 = e_sb[:].rearrange("p (w t) -> p w t", t=S)
                    z = smp.tile([64, 32], f32)
                    nc.vector.tensor_reduce(z[:], e3, axis=mybir.AxisListType.X,
                                            op=mybir.AluOpType.add)
                    rcp = smp.tile([64, 32], f32)
                    nc.vector.reciprocal(rcp[:], z[:])
                    nc.vector.tensor_mul(
                        p_sb[:, bass.ts(wh, 384)].rearrange(
                            "p (w t) -> p w t", t=S),
                        e3, rcp[:].broadcast_to([64, 32, S]))
            if debug:
                nc.sync.dma_start(dbg_p[:], p_sb[:])

            nc.sync.dma_start(dram_p[:], p_sb[:])
            # p_hw[(wb, h), (w', t)] <- dram_p[h, (w, t)], w = wb*32 + w'
            for wb in range(2):
                ring(wb).dma_start(
                    p_hw[wb * 64:(wb + 1) * 64, :],
                    dram_p[:, wb * WP * S:(wb + 1) * WP * S])
            if debug:
                nc.sync.dma_start(dbg_ph[:], p_hw[:])

            # ---- stage B: aligned multiply, t-reduce, transpose, channel mix
            with ExitStack() as bctx:
                pwp = bctx.enter_context(tc.tile_pool(name="pw", bufs=2))
                psT = bctx.enter_context(
                    tc.tile_pool(name="psT", bufs=4, space="PSUM"))
                psY = bctx.enter_context(
                    tc.tile_pool(name="psY", bufs=4, space="PSUM"))
                qtp = bctx.enter_context(tc.tile_pool(name="qt", bufs=4))
                for ch in range(NCH):
                    pw = pwp.tile([128, CHX], bf16)   # (w' 8, c 64, t 12)
                    pw4 = pw[:].rearrange("p (w c t) -> p w c t", w=8, t=S)
                    x4 = x2_sb[:, bass.ts(ch, CHX)].rearrange(
                        "p (w c t) -> p w c t", w=8, t=S)
                    p4 = p_hw[:, ch * 8 * S:(ch + 1) * 8 * S].rearrange(
                        "p (w o t) -> p w o t", o=1, t=S).broadcast_to(
                        [128, 8, 64, S])
                    nc.vector.tensor_mul(pw4, x4, p4)
                    # q[:, ch block] = sum_t pw  (dense innermost-t reduce)
                    qv = q_sb[:, bass.ts(ch, 512)]
                    nc.vector.tensor_reduce(
                        qv.rearrange("p (w c) -> p w c", w=8),
                        pw4, axis=mybir.AxisListType.X, op=mybir.AluOpType.add)
                    for i in range(4):
                        jj = ch * 4 + i
                        t_ps = psT.tile([128, 128], f32)
                        nc.tensor.transpose(
                            t_ps[:], q_sb[:, jj * 128:(jj + 1) * 128],
                            ident[:])
                        qt = qtp.tile([128, 128], bf16)
                        nc.scalar.activation(
                            qt[:], t_ps[:],
                            mybir.ActivationFunctionType.Identity)
                        y_ps = psY.tile([128, 128], f32)
                        nc.tensor.matmul(y_ps[:], wvp[:], qt[:],
                                         start=True, stop=True)
                        nc.scalar.activation(
                            y_sb[:, jj * 128:(jj + 1) * 128], y_ps[:],
                            mybir.ActivationFunctionType.Identity,
                            bias=bvp[:, 0:1])
                if debug:
                    nc.sync.dma_start(dbg_q[:], q_sb[:])
            nc.sync.dma_start(y_out[:], y_sb[:])

    nc.compile()
    return nc


def _prep_weights(w1, b1, w2, b2):
    import ml_dtypes
    bf = ml_dtypes.bfloat16
    w1f = w1[:, :, 0, 0].astype(np.float32)
    wk, wv = w1f[64:128], w1f[128:192]
    bv = b1[128:192].astype(np.float32)
    w2k = w2[0, 64:128].astype(np.float32)                     # [c,5,5]
    u = np.tensordot(w2k, wk, axes=([0], [0])).reshape(NT, C)  # [25, 64]
    u2 = np.zeros((128, 50), np.float32)
    u2[0:64, 0:25] = u.T
    u2[64:128, 25:50] = u.T
    # s5[(dhp, h_in), (k, h_out)] = 1 iff h_in == h_out + (2k+dhp) - 2
    s5 = np.zeros((2, 64, 3, 64), np.float32)
    for dh in range(5):
        k, dhp = dh // 2, dh % 2
        for ho in range(64):
            hi = ho + dh - 2
            if 0 <= hi < 64:
                s5[dhp, hi, k, ho] = 1.0
    s5 = s5.reshape(128, 3 * 64)
    ident = np.eye(128, dtype=np.float32)
    # wvp[(w'a, c), (w'b, co)] = wv[co, c] * [w'a == w'b]
    wvp = np.zeros((2, 64, 2, 64), np.float32)
    wvp[0, :, 0, :] = wv.T
    wvp[1, :, 1, :] = wv.T
    wvp = wvp.reshape(128, 128)
    bvp = np.concatenate([bv, bv]).reshape(128, 1).astype(np.float32)
    return (u2.astype(bf), s5.astype(bf), ident, wvp.astype(bf), bvp)


def _run(x, w1, b1, w2, b2, trace=False):
    import ml_dtypes
    from concourse.bass_utils import run_bass_kernel_spmd
    bf = ml_dtypes.bfloat16
    if "nc" not in _cache:
        _cache["nc"] = _build_program()
    nc = _cache["nc"]
    u2, s5, ident, wvp, bvp = _prep_weights(w1, b1, w2, b2)
    in_maps = []
    for b in range(B):
        xb = np.ascontiguousarray(
            x[b].reshape(C, HB, HH, W * S).transpose(1, 0, 2, 3)
            .reshape(128, FREE)).astype(bf)
        # x2 [(wb, h), (w', c, t)]
        x2b = np.ascontiguousarray(
            x[b].reshape(C, H, 2, WP, S).transpose(2, 1, 3, 0, 4)
            .reshape(128, FREE)).astype(bf)
        in_maps.append({"x": xb, "x2": x2b, "u2": u2, "s5": s5,
                        "ident": ident, "wvp": wvp, "bvp": bvp})
    res = run_bass_kernel_spmd(nc, in_maps, core_ids=list(range(8)), trace=trace)
    out = np.empty((B, C, H, W, S), np.float32)
    for b in range(B):
        # y [128=(w'a, co), 2048=(jj, wb, h)]
        yb = res.results[b]["y"].reshape(2, C, NTL, 2, H)
        # out[co, h, w = wb*32 + jj*2 + w'a]
        yb = yb.transpose(1, 4, 3, 2, 0).reshape(C, H, W)
        out[b] = yb[..., None]
    return out, res


def kernel(x, w1, b1, w2, b2):
    out, _ = _run(x, w1, b1, w2, b2, trace=False)
    return out
